# revision 1
# baseline (speedup 1.0000x reference)
"""HGConv fused kernel for one TRN2 chip (8 NeuronCores), SPMD via Bass/Tile.

Hardcoded for M=16384 nodes, E=4096 hyperedges, D=300, N_CAT=3, 8 cores.

  - Shard the node axis m: core c gets node_feats rows [2048c, 2048(c+1))
    and the matching inc_mat rows.  Phase 1 computes the partial
    IX_c = inc_c.T @ X_c (4096, 300) with inc tiles stationary on the PE.
  - ReduceScatter(add) turns the partials into the true IX = inc.T @ X,
    e-sharded: core c owns edges [512c, 512(c+1)).
  - Local tail per core: edge_att = IX @ W_att (reassociated from
    inc.T @ (X @ W_att)), softmax over d, ef = (IX * attn) @ W_proj,
    residual mix with edge_feats, scores = ef2 @ ec_W_att, locally
    stabilized exp, G = ef2 @ ec_W_proj, partial pooled vector
    p2 = sum_e exp_e * G[e, :].
  - AllGather of the per-core (p2, z, m) partials (304 floats); every core
    redundantly combines them (global softmax over edges) and applies the
    two tiny projections to produce the (3,) logits.
"""

import sys

for _p in ("/opt/trn_rl_repo", "/opt/pypackages"):
    if _p not in sys.path:
        sys.path.append(_p)

import numpy as np

import concourse.bacc as bacc
import concourse.tile as tile
from concourse import masks, mybir
from concourse.bass_utils import run_bass_kernel_spmd

F32 = mybir.dt.float32
F32R = mybir.dt.float32r
BF16 = mybir.dt.bfloat16
AX = mybir.AxisListType
OP = mybir.AluOpType
AF = mybir.ActivationFunctionType

NCORES = 8
M, E, D, NCAT = 16384, 4096, 300, 3
M_SH = M // NCORES          # 2048 nodes per core
E_SH = E // NCORES          # 512 edges per core (tail shard)
MT = M_SH // 128            # 16 m-tiles per core
ET_SH = E_SH // 128         # 4 e-tiles per core
DCH = (128, 128, 44)        # d split into partition chunks
DOF = (0, 128, 256)
E_BLK = 1024                # phase-1 e block (8 psum banks)
N_EBLK = E // E_BLK
E_SUB = E_BLK // 128


def _build(alpha: float, mode: str):
    nc = bacc.Bacc("TRN2", target_bir_lowering=False, debug=False,
                   num_devices=NCORES)
    in_dt = BF16 if mode == "bf16" else F32
    x_d = nc.dram_tensor("x", [M_SH, D], in_dt, kind="ExternalInput")
    inc_d = nc.dram_tensor("inc", [M_SH, E], in_dt, kind="ExternalInput")
    ef_d = nc.dram_tensor("efeat", [E_SH, D], F32, kind="ExternalInput")
    watt_d = nc.dram_tensor("watt", [D, D], F32, kind="ExternalInput")
    wproj_d = nc.dram_tensor("wproj", [D, D], F32, kind="ExternalInput")
    ecwatt_d = nc.dram_tensor("ecwatt", [D, 1], F32, kind="ExternalInput")
    ecwproj_d = nc.dram_tensor("ecwproj", [D, D], F32, kind="ExternalInput")
    ecb_d = nc.dram_tensor("ecb", [D], F32, kind="ExternalInput")
    fcw_d = nc.dram_tensor("fcw", [D, NCAT], F32, kind="ExternalInput")
    fcb_d = nc.dram_tensor("fcb", [NCAT], F32, kind="ExternalInput")
    out_d = nc.dram_tensor("out", [1, NCAT], F32, kind="ExternalOutput")

    groups = [list(range(NCORES))]

    rdt = {"f32": F32, "f32r": F32R, "bf16": BF16}[mode]
    e_blk = 2048 if mode == "bf16" else 1024
    n_eblk = E // e_blk
    e_sub = e_blk // 128

    def mm(out, lhsT, rhs, start, stop):
        nc.tensor.matmul(out, lhsT, rhs, start=start, stop=stop)

    def rsrc(ap):
        return ap.bitcast(F32R) if mode == "f32r" else ap

    with tile.TileContext(nc) as tc, \
         tc.tile_pool(name="sb", bufs=1) as sb, \
         tc.tile_pool(name="dram", bufs=1, space="DRAM") as dram:

        p_chunks = [dram.tile([1024, D], F32, name=f"p_chunk{k}")
                    for k in range(4)]          # RS inputs (partial IX)
        r_ks = [dram.tile([128, D], F32, name=f"r_k{k}")
                for k in range(4)]              # RS outputs (my 128 edges)
        pk_dram = dram.tile([304], F32)         # AG input
        gath = dram.tile([NCORES, 304], F32)    # AG output

        # ---------- phase 1: IX partial = inc_c.T @ X_c ----------
        x_sb = sb.tile([128, MT, D], rdt)
        nc.sync.dma_start(x_sb[:], rsrc(x_d.ap().rearrange("(t p) d -> p t d",
                                                           p=128)))
        with tc.tile_pool(name="incp", bufs=MT + 8) as incp, \
             tc.tile_pool(name="stg", bufs=8) as stg, \
             tc.tile_pool(name="pp1", bufs=8, space="PSUM") as pp1:
            for blk in range(n_eblk):
                inc_sb = [incp.tile([128, e_blk], rdt, tag="inc",
                                    name=f"inc_b{blk}_m{m}")
                          for m in range(MT)]
                for m in range(MT):
                    eng = nc.sync if m % 2 == 0 else nc.scalar
                    eng.dma_start(
                        inc_sb[m][:],
                        rsrc(inc_d[m * 128:(m + 1) * 128,
                                   blk * e_blk:(blk + 1) * e_blk]))
                for es in range(e_sub):
                    acc = pp1.tile([128, D], F32, tag="p1")
                    for m in range(MT):
                        mm(acc[:], inc_sb[m][:, es * 128:(es + 1) * 128],
                           x_sb[:, m, :], start=(m == 0), stop=(m == MT - 1))
                    stage = stg.tile([128, D], F32, tag="stage",
                                     name=f"stage_{blk}_{es}")
                    nc.vector.tensor_copy(stage[:], acc[:])
                    eg = blk * e_blk + es * 128        # global edge offset
                    k, row = eg // 1024, eg % 1024
                    nc.gpsimd.dma_start(p_chunks[k][row:row + 128, :],
                                        stage[:])
                    # phase 2 (chunked, overlapped): as soon as chunk k is
                    # fully written, ReduceScatter it while the next block
                    # computes.
                    if row == 1024 - 128:
                        nc.gpsimd.collective_compute(
                            "ReduceScatter", OP.add, replica_groups=groups,
                            ins=[p_chunks[k].opt()], outs=[r_ks[k].opt()])

        # ---------- small weights / constants ----------
        watt_sb = sb.tile([128, 3, D], F32)
        wproj_sb = sb.tile([128, 3, D], F32)
        ecwproj_sb = sb.tile([128, 3, D], F32)
        fcw_sb = sb.tile([128, 3, NCAT], F32)
        ecwatt_sb = sb.tile([128, 3, 1], F32)
        for i, (c, o) in enumerate(zip(DCH, DOF)):
            nc.sync.dma_start(watt_sb[:c, i, :], watt_d[o:o + c, :])
            nc.sync.dma_start(wproj_sb[:c, i, :], wproj_d[o:o + c, :])
            nc.sync.dma_start(ecwproj_sb[:c, i, :], ecwproj_d[o:o + c, :])
            nc.sync.dma_start(fcw_sb[:c, i, :], fcw_d[o:o + c, :])
            nc.sync.dma_start(ecwatt_sb[:c, i, :], ecwatt_d[o:o + c, :])
        ecb_sb = sb.tile([1, D], F32)
        nc.sync.dma_start(ecb_sb[:], ecb_d.ap().rearrange("(o d) -> o d", o=1))
        fcb_sb = sb.tile([1, NCAT], F32)
        nc.sync.dma_start(fcb_sb[:], fcb_d.ap().rearrange("(o d) -> o d", o=1))
        ident = sb.tile([128, 128], F32)
        masks.make_identity(nc, ident[:])
        efeat_sb = sb.tile([128, ET_SH, D], F32)
        nc.sync.dma_start(efeat_sb[:],
                          ef_d.ap().rearrange("(t p) d -> p t d", p=128))

        # ---------- phase 3: local tail on this core's 512 edges ----------
        ix_sb = sb.tile([128, ET_SH, D], F32)
        for k in range(4):
            nc.sync.dma_start(ix_sb[:, k, :], r_ks[k][:])

        with tc.tile_pool(name="pp2", bufs=4, space="PSUM") as pp:

            def transpose_512xD(src_sb, dstT_sb):
                # src (128, 4, 300) [e-part] -> dstT (128, 3, 512) [d-part]
                for et in range(ET_SH):
                    for i, (c, o) in enumerate(zip(DCH, DOF)):
                        tp = pp.tile([128, 128], F32, tag="ps")
                        nc.tensor.transpose(tp[:c, :128],
                                            src_sb[:, et, o:o + c], ident[:])
                        nc.scalar.copy(
                            dstT_sb[:c, i, et * 128:(et + 1) * 128],
                            tp[:c, :128])

            ixT_sb = sb.tile([128, 3, E_SH], F32)
            transpose_512xD(ix_sb, ixT_sb)

            # edge_att = IX @ W_att; softmax over d; ef = IX * attn
            ef2_sb = sb.tile([128, ET_SH, D], F32)
            stat_sb = sb.tile([128, ET_SH, 4], F32)
            for et in range(ET_SH):
                att = pp.tile([128, D], F32, tag="ps")
                for i, c in enumerate(DCH):
                    mm(att[:], ixT_sb[:c, i, et * 128:(et + 1) * 128],
                       watt_sb[:c, i, :], start=(i == 0), stop=(i == 2))
                nmax = stat_sb[:, et, 0:1]
                nc.vector.tensor_reduce(nmax, att[:], axis=AX.X, op=OP.max,
                                        negate=True)
                ex = pp.tile([128, D], F32, tag="ps")
                rsum = stat_sb[:, et, 1:2]
                nc.scalar.activation(ex[:], att[:], AF.Exp, bias=nmax,
                                     scale=1.0, accum_out=rsum)
                rcp = stat_sb[:, et, 2:3]
                nc.vector.reciprocal(rcp, rsum)
                nc.vector.scalar_tensor_tensor(
                    ef2_sb[:, et, :], ex[:], rcp, ix_sb[:, et, :],
                    op0=OP.mult, op1=OP.mult)

            efT_sb = sb.tile([128, 3, E_SH], F32)
            transpose_512xD(ef2_sb, efT_sb)

            # ef2 = alpha * edge_feats + (1 - alpha) * (ef @ W_proj)
            efs_sb = sb.tile([128, ET_SH, D], F32)
            for et in range(ET_SH):
                prj = pp.tile([128, D], F32, tag="ps")
                for i, c in enumerate(DCH):
                    mm(prj[:], efT_sb[:c, i, et * 128:(et + 1) * 128],
                       wproj_sb[:c, i, :], start=(i == 0), stop=(i == 2))
                nc.scalar.mul(efs_sb[:, et, :], efeat_sb[:, et, :],
                              float(alpha))
                nc.vector.scalar_tensor_tensor(
                    ef2_sb[:, et, :], prj[:], float(1.0 - alpha),
                    efs_sb[:, et, :], op0=OP.mult, op1=OP.add)

            ef2T_sb = sb.tile([128, 3, E_SH], F32)
            transpose_512xD(ef2_sb, ef2T_sb)

            # scores (1, 512); locally stabilized exp weights
            sc = pp.tile([1, E_SH], F32, tag="ps")
            for i, c in enumerate(DCH):
                mm(sc[:], ecwatt_sb[:c, i, :], ef2T_sb[:c, i, :],
                   start=(i == 0), stop=(i == 2))
            one_sb = sb.tile([1, 520], F32)
            nloc = one_sb[:, 512:513]
            nc.vector.tensor_reduce(nloc, sc[:], axis=AX.X, op=OP.max,
                                    negate=True)
            expw = one_sb[:, 0:512]
            zloc = one_sb[:, 513:514]
            nc.scalar.activation(expw, sc[:], AF.Exp, bias=nloc, scale=1.0,
                                 accum_out=zloc)
            mloc = one_sb[:, 514:515]
            nc.scalar.mul(mloc, nloc, -1.0)

            expcol_sb = sb.tile([128, ET_SH], F32)
            for et in range(ET_SH):
                tc1 = pp.tile([128, 1], F32, tag="ps")
                nc.tensor.transpose(tc1[:],
                                    expw[0:1, et * 128:(et + 1) * 128],
                                    ident[0:1, 0:1])
                nc.scalar.copy(expcol_sb[:, et:et + 1], tc1[:])

            # G = ef2 @ ec_W_proj ; p2 = expw^T @ G (pooling + proj folded)
            g_sb = sb.tile([128, ET_SH, D], F32)
            for et in range(ET_SH):
                g = pp.tile([128, D], F32, tag="ps")
                for i, c in enumerate(DCH):
                    mm(g[:], ef2T_sb[:c, i, et * 128:(et + 1) * 128],
                       ecwproj_sb[:c, i, :], start=(i == 0), stop=(i == 2))
                nc.scalar.copy(g_sb[:, et, :], g[:])
            p2 = pp.tile([1, D], F32, tag="acc")
            for et in range(ET_SH):
                mm(p2[:], expcol_sb[:, et:et + 1], g_sb[:, et, :],
                   start=(et == 0), stop=(et == ET_SH - 1))

            pk_sb = sb.tile([1, 304], F32)
            nc.scalar.copy(pk_sb[:, 0:D], p2[:])
            nc.scalar.copy(pk_sb[:, 300:301], zloc)
            nc.scalar.copy(pk_sb[:, 301:302], mloc)
            nc.vector.memset(pk_sb[:, 302:304], 0.0)
            nc.sync.dma_start(pk_dram[:], pk_sb[0:1, :])

            # ---------- phase 4: AllGather + redundant epilogue ----------
            nc.gpsimd.collective_compute(
                "AllGather", OP.bypass, replica_groups=groups,
                ins=[pk_dram.opt()], outs=[gath.opt()])

            grow = sb.tile([1, NCORES, 304], F32)
            nc.sync.dma_start(
                grow[:], gath[:].rearrange("c k -> (c k)").rearrange(
                    "(o c k) -> o c k", o=1, c=NCORES))
            g8 = sb.tile([NCORES, 304], F32)
            nc.sync.dma_start(g8[:], gath[:])

            eps_sb = sb.tile([1, 16], F32)
            ngmax = eps_sb[:, 0:1]
            nc.vector.tensor_reduce(ngmax, grow[:, :, 301], axis=AX.X,
                                    op=OP.max, negate=True)
            scal_row = eps_sb[:, 1:9]
            nc.scalar.activation(scal_row, grow[:, :, 301], AF.Exp,
                                 bias=ngmax, scale=1.0)
            sccol = pp.tile([NCORES, 1], F32, tag="ps")
            nc.tensor.transpose(sccol[:], scal_row, ident[0:1, 0:1])
            sccol_sb = sb.tile([NCORES, 1], F32)
            nc.scalar.copy(sccol_sb[:], sccol[:])
            comb = pp.tile([1, 304], F32, tag="ps")
            nc.tensor.matmul(comb[:], sccol_sb[:], g8[:], start=True,
                             stop=True)
            rz = eps_sb[:, 9:10]
            nc.vector.reciprocal(rz, comb[:, 300:301])
            pooled_sb = sb.tile([1, D], F32)
            nc.vector.tensor_scalar_mul(pooled_sb[:], comb[:, 0:D], rz)
            nc.vector.tensor_add(pooled_sb[:], pooled_sb[:], ecb_sb[:])

            ocol_sb = sb.tile([128, 3], F32)
            for i, (c, o) in enumerate(zip(DCH, DOF)):
                tpc = pp.tile([128, 1], F32, tag="ps")
                nc.tensor.transpose(tpc[:c, :], pooled_sb[0:1, o:o + c],
                                    ident[0:1, 0:1])
                nc.scalar.copy(ocol_sb[:c, i:i + 1], tpc[:c, :])
            lg = pp.tile([1, NCAT], F32, tag="acc")
            for i, c in enumerate(DCH):
                nc.tensor.matmul(lg[:], ocol_sb[:c, i:i + 1],
                                 fcw_sb[:c, i, :], start=(i == 0),
                                 stop=(i == 2))
            logit_sb = sb.tile([1, NCAT], F32)
            nc.vector.tensor_add(logit_sb[:], lg[:], fcb_sb[:])
            nc.sync.dma_start(out_d[:], logit_sb[:])

    nc.compile()
    return nc


_CACHE = {}


def get_nc(alpha: float, mode: str = "f32r"):
    key = (alpha, mode)
    if key not in _CACHE:
        _CACHE[key] = _build(alpha, mode)
    return _CACHE[key]


def make_in_maps(node_feats, edge_feats, inc_mat, W_att, W_proj,
                 ec_W_att, ec_W_proj, ec_b_proj, fc_W, fc_b, mode="f32r"):
    cc = lambda a: np.ascontiguousarray(np.asarray(a, np.float32))
    node_feats, inc_mat, edge_feats = cc(node_feats), cc(inc_mat), cc(edge_feats)
    if mode == "bf16":
        import ml_dtypes
        node_feats = node_feats.astype(ml_dtypes.bfloat16)
        inc_mat = inc_mat.astype(ml_dtypes.bfloat16)
    common = dict(watt=cc(W_att), wproj=cc(W_proj),
                  ecwatt=cc(ec_W_att).reshape(D, 1), ecwproj=cc(ec_W_proj),
                  ecb=cc(ec_b_proj), fcw=cc(fc_W), fcb=cc(fc_b))
    in_maps = []
    for c in range(NCORES):
        # under chunked RS, core c owns edges {1024k + 128c .. +128} k=0..3
        eidx = np.concatenate([np.arange(1024 * k + 128 * c,
                                         1024 * k + 128 * (c + 1))
                               for k in range(4)])
        in_maps.append(dict(
            x=node_feats[c * M_SH:(c + 1) * M_SH],
            inc=np.ascontiguousarray(inc_mat[c * M_SH:(c + 1) * M_SH]),
            efeat=np.ascontiguousarray(edge_feats[eidx]),
            **common))
    return in_maps


def kernel(node_feats, edge_feats, inc_mat, W_att, W_proj, alpha,
           ec_W_att, ec_W_proj, ec_b_proj, fc_W, fc_b,
           mode="f32r", trace=False):
    nc = get_nc(float(np.asarray(alpha)), mode)
    in_maps = make_in_maps(node_feats, edge_feats, inc_mat, W_att, W_proj,
                           ec_W_att, ec_W_proj, ec_b_proj, fc_W, fc_b,
                           mode=mode)
    res = run_bass_kernel_spmd(nc, in_maps, list(range(NCORES)), trace=trace)
    kernel.last_results = res
    return res.results[0]["out"].reshape(NCAT).astype(np.float32)



# revision 5
# speedup vs baseline: 2.6075x; 2.6075x over previous
"""HGConv fused kernel for one TRN2 chip (8 NeuronCores), SPMD via Bass/Tile.

Hardcoded for M=16384 nodes, E=4096 hyperedges, D=300, N_CAT=3, 8 cores.

Strategy (edge-sharded, zero collectives):
  - Core c owns hyperedges [512c, 512(c+1)).  It loads the FULL node
    matrix X (16384, 300) plus its own 512-column slice of inc, both in
    fp16 (halves HBM traffic; final rel err ~8e-4), and computes
    IX = inc_c.T @ X with the full m=16384 contraction locally — no
    ReduceScatter, no AllGather, no cross-core barrier at all.
  - Host pre-tiles both operands to [p, t, ...] layout so every DMA is
    a contiguous multi-KB line per partition; inc streams through SBUF
    in 8 chunks (double+ buffered) overlapping the PE accumulation.
  - Local tail on the 512 edges: edge_att = IX @ W_att, softmax over d,
    ef = (IX * attn) @ W_proj, residual mix with alpha-prescaled
    edge_feats, scores = ef2 @ ec_W_att, locally-stabilized exp
    weights, p2 = sum_e w_e * ef2[e,:], then the two tiny projections
    r = (p2 @ ec_W_proj) @ fc_W on device.
  - Each core outputs just 8 floats: [r(3), z, m, pad].  The host
    combines the 8 partials (global softmax over edges is a weighted
    sum of the locally-normalized partials) and adds the constant
    bias term ec_b @ fc_W + fc_b.
"""

import sys

for _p in ("/opt/trn_rl_repo", "/opt/pypackages"):
    if _p not in sys.path:
        sys.path.append(_p)

import numpy as np

import concourse.bacc as bacc
import concourse.tile as tile
from concourse import masks, mybir
from concourse.bass_utils import run_bass_kernel_spmd

F32 = mybir.dt.float32
F32R = mybir.dt.float32r
F16 = mybir.dt.float16
BF16 = mybir.dt.bfloat16
AX = mybir.AxisListType
OP = mybir.AluOpType
AF = mybir.ActivationFunctionType

NCORES = 8
M, E, D, NCAT = 16384, 4096, 300, 3
E_SH = E // NCORES          # 512 edges per core
ET = E_SH // 128            # 4 e-tiles per core
T = M // 128                # 128 m-tiles (full contraction on every core)
NCH = 8                     # inc/x streaming chunks
TC = T // NCH               # 16 m-tiles per chunk
DCH = (128, 128, 44)        # d split into partition chunks
DOF = (0, 128, 256)


def _build(alpha: float, mode: str):
    nc = bacc.Bacc("TRN2", target_bir_lowering=False, debug=False,
                   num_devices=NCORES)
    in_dt = {"f16": F16, "bf16": BF16, "f32r": F32}[mode]
    x_d = nc.dram_tensor("x", [128, T, D], in_dt, kind="ExternalInput")
    inc_d = nc.dram_tensor("inc", [128, T, E_SH], in_dt, kind="ExternalInput")
    ef_d = nc.dram_tensor("efeat", [E_SH, D], F32, kind="ExternalInput")
    watt_d = nc.dram_tensor("watt", [D, D], F32, kind="ExternalInput")
    wproj_d = nc.dram_tensor("wproj", [D, D], F32, kind="ExternalInput")
    ecwatt_d = nc.dram_tensor("ecwatt", [D, 1], F32, kind="ExternalInput")
    ecwproj_d = nc.dram_tensor("ecwproj", [D, D], F32, kind="ExternalInput")
    fcw_d = nc.dram_tensor("fcw", [D, NCAT], F32, kind="ExternalInput")
    out_d = nc.dram_tensor("out", [1, 8], F32, kind="ExternalOutput")

    def rsrc(ap):
        return ap.bitcast(F32R) if mode == "f32r" else ap

    def mm(out, lhsT, rhs, start, stop):
        nc.tensor.matmul(out, lhsT, rhs, start=start, stop=stop)

    with tile.TileContext(nc) as tc, \
         tc.tile_pool(name="sb", bufs=1) as sb, \
         tc.tile_pool(name="incp", bufs=3) as incp, \
         tc.tile_pool(name="pacc", bufs=1, space="PSUM") as pacc, \
         tc.tile_pool(name="pp", bufs=4, space="PSUM") as pp:

        # ---------- resident X + streamed inc; IX = inc_c.T @ X ----------
        x_sb = sb.tile([128, T, D], in_dt)
        accs = [pacc.tile([128, D], F32, tag=f"acc{et}", name=f"acc{et}")
                for et in range(ET)]
        for ch in range(NCH):
            nc.sync.dma_start(x_sb[:, ch * TC:(ch + 1) * TC, :],
                              rsrc(x_d[:, ch * TC:(ch + 1) * TC, :]))
        # small weights / constants (issued early, consumed by the tail)
        watt_sb = sb.tile([128, 3, D], F32)
        wproj_sb = sb.tile([128, 3, D], F32)
        ecwproj_sb = sb.tile([128, 3, D], F32)
        fcw_sb = sb.tile([128, 3, NCAT], F32)
        ecwatt_sb = sb.tile([128, 3, 1], F32)
        for i, (c, o) in enumerate(zip(DCH, DOF)):
            nc.sync.dma_start(watt_sb[:c, i, :], watt_d[o:o + c, :])
            nc.sync.dma_start(wproj_sb[:c, i, :], wproj_d[o:o + c, :])
            nc.sync.dma_start(ecwproj_sb[:c, i, :], ecwproj_d[o:o + c, :])
            nc.sync.dma_start(fcw_sb[:c, i, :], fcw_d[o:o + c, :])
            nc.sync.dma_start(ecwatt_sb[:c, i, :], ecwatt_d[o:o + c, :])
        efeat_sb = sb.tile([128, ET, D], F32)   # pre-scaled by alpha on host
        nc.sync.dma_start(efeat_sb[:],
                            ef_d.ap().rearrange("(t p) d -> p t d", p=128))
        ident = sb.tile([128, 128], F32)
        masks.make_identity(nc, ident[:])

        for ch in range(NCH):
            inc_sb = incp.tile([128, TC, E_SH], in_dt, tag="inc",
                               name=f"inc{ch}")
            eng = nc.scalar if ch % 2 == 0 else nc.gpsimd
            eng.dma_start(inc_sb[:], rsrc(inc_d[:, ch * TC:(ch + 1) * TC, :]))
            for tt in range(TC):
                t = ch * TC + tt
                for et in range(ET):
                    mm(accs[et][:], inc_sb[:, tt, et * 128:(et + 1) * 128],
                       x_sb[:, t, :], start=(t == 0), stop=(t == T - 1))

        # ---------- local tail on this core's 512 edges ----------
        ix_sb = sb.tile([128, ET, D], F32)
        for et in range(ET):
            nc.vector.tensor_copy(ix_sb[:, et, :], accs[et][:])

        def transpose_512xD(src_sb, dstT_sb):
            # src (128, 4, 300) [e-part] -> dstT (128, 3, 512) [d-part]
            for et in range(ET):
                for i, (c, o) in enumerate(zip(DCH, DOF)):
                    tp = pp.tile([128, 128], F32, tag="ps")
                    nc.tensor.transpose(tp[:c, :128],
                                        src_sb[:, et, o:o + c], ident[:])
                    nc.scalar.copy(dstT_sb[:c, i, et * 128:(et + 1) * 128],
                                   tp[:c, :128])

        ixT_sb = sb.tile([128, 3, E_SH], F32)
        transpose_512xD(ix_sb, ixT_sb)

        # edge_att = IX @ W_att; softmax over d; ef = IX * attn
        ef2_sb = sb.tile([128, ET, D], F32)
        stat_sb = sb.tile([128, ET, 4], F32)
        for et in range(ET):
            att = pp.tile([128, D], F32, tag="ps")
            for i, c in enumerate(DCH):
                mm(att[:], ixT_sb[:c, i, et * 128:(et + 1) * 128],
                   watt_sb[:c, i, :], start=(i == 0), stop=(i == 2))
            nmax = stat_sb[:, et, 0:1]
            nc.vector.tensor_reduce(nmax, att[:], axis=AX.X, op=OP.max,
                                    negate=True)
            ex = pp.tile([128, D], F32, tag="ps")
            rsum = stat_sb[:, et, 1:2]
            nc.scalar.activation(ex[:], att[:], AF.Exp, bias=nmax,
                                 scale=1.0, accum_out=rsum)
            rcp = stat_sb[:, et, 2:3]
            nc.vector.reciprocal(rcp, rsum)
            nc.vector.scalar_tensor_tensor(
                ef2_sb[:, et, :], ex[:], rcp, ix_sb[:, et, :],
                op0=OP.mult, op1=OP.mult)

        efT_sb = sb.tile([128, 3, E_SH], F32)
        transpose_512xD(ef2_sb, efT_sb)

        # ef2 = alpha * edge_feats + (1 - alpha) * (ef @ W_proj)
        for et in range(ET):
            prj = pp.tile([128, D], F32, tag="ps")
            for i, c in enumerate(DCH):
                mm(prj[:], efT_sb[:c, i, et * 128:(et + 1) * 128],
                   wproj_sb[:c, i, :], start=(i == 0), stop=(i == 2))
            nc.vector.scalar_tensor_tensor(
                ef2_sb[:, et, :], prj[:], float(1.0 - alpha),
                efeat_sb[:, et, :], op0=OP.mult, op1=OP.add)

        ef2T_sb = sb.tile([128, 3, E_SH], F32)
        transpose_512xD(ef2_sb, ef2T_sb)

        # scores (1, 512); locally stabilized exp weights
        sc = pp.tile([1, E_SH], F32, tag="ps")
        for i, c in enumerate(DCH):
            mm(sc[:], ecwatt_sb[:c, i, :], ef2T_sb[:c, i, :],
               start=(i == 0), stop=(i == 2))
        one_sb = sb.tile([1, 520], F32)
        nloc = one_sb[:, 512:513]
        nc.vector.tensor_reduce(nloc, sc[:], axis=AX.X, op=OP.max,
                                negate=True)
        expw = one_sb[:, 0:512]
        zloc = one_sb[:, 513:514]
        nc.scalar.activation(expw, sc[:], AF.Exp, bias=nloc, scale=1.0,
                             accum_out=zloc)
        mloc = one_sb[:, 514:515]
        nc.scalar.mul(mloc, nloc, -1.0)

        expcol_sb = sb.tile([128, ET], F32)
        for et in range(ET):
            tc1 = pp.tile([128, 1], F32, tag="ps")
            nc.tensor.transpose(tc1[:], expw[0:1, et * 128:(et + 1) * 128],
                                ident[0:1, 0:1])
            nc.scalar.copy(expcol_sb[:, et:et + 1], tc1[:])

        # p2 = sum_e w_e * ef2[e, :]  (pooling with unnormalized weights)
        p2 = pp.tile([1, D], F32, tag="ps")
        for et in range(ET):
            mm(p2[:], expcol_sb[:, et:et + 1], ef2_sb[:, et, :],
               start=(et == 0), stop=(et == ET - 1))
        p2_sb = sb.tile([1, D], F32)
        nc.scalar.copy(p2_sb[:], p2[:])

        def rowvec_project(row_sb, w_sb, out_psum, nout):
            # out (1, nout) = row (1, 300) @ W (300, nout), W d-chunked
            col_sb = sb.tile([128, 3, 1], F32, name=f"col_{nout}")
            for i, (c, o) in enumerate(zip(DCH, DOF)):
                tpc = pp.tile([128, 1], F32, tag="ps")
                nc.tensor.transpose(tpc[:c, :], row_sb[0:1, o:o + c],
                                    ident[0:1, 0:1])
                nc.scalar.copy(col_sb[:c, i, :], tpc[:c, :])
            for i, c in enumerate(DCH):
                mm(out_psum[:], col_sb[:c, i, :], w_sb[:c, i, :],
                   start=(i == 0), stop=(i == 2))

        # r = (p2 @ ec_W_proj) @ fc_W  (both linear, biases folded on host)
        q = pp.tile([1, D], F32, tag="ps")
        rowvec_project(p2_sb, ecwproj_sb, q, D)
        q_sb = sb.tile([1, D], F32)
        nc.scalar.copy(q_sb[:], q[:])
        r = pp.tile([1, NCAT], F32, tag="ps")
        rowvec_project(q_sb, fcw_sb, r, NCAT)

        out_sb = sb.tile([1, 8], F32)
        nc.vector.memset(out_sb[:], 0.0)
        nc.scalar.copy(out_sb[:, 0:NCAT], r[:])
        nc.scalar.copy(out_sb[:, 3:4], zloc)
        nc.scalar.copy(out_sb[:, 4:5], mloc)
        nc.sync.dma_start(out_d[:], out_sb[:])

    nc.compile()
    return nc


_CACHE = {}


def get_nc(alpha: float, mode: str = "f16"):
    key = (alpha, mode)
    if key not in _CACHE:
        _CACHE[key] = _build(alpha, mode)
    return _CACHE[key]


def make_in_maps(node_feats, edge_feats, inc_mat, W_att, W_proj, alpha,
                 ec_W_att, ec_W_proj, fc_W, mode="f16"):
    cc = lambda a: np.ascontiguousarray(np.asarray(a, np.float32))
    ndt = {"f16": np.float16, "f32r": np.float32}.get(mode)
    if ndt is None:
        import ml_dtypes
        ndt = ml_dtypes.bfloat16
    x = np.asarray(node_feats, np.float32).astype(ndt)
    xt = np.ascontiguousarray(x.reshape(T, 128, D).transpose(1, 0, 2))
    inc = np.asarray(inc_mat, np.float32).astype(ndt)
    ef_scaled = np.asarray(edge_feats, np.float32) * np.float32(alpha)
    common = dict(x=xt, watt=cc(W_att), wproj=cc(W_proj),
                  ecwatt=cc(ec_W_att).reshape(D, 1), ecwproj=cc(ec_W_proj),
                  fcw=cc(fc_W))
    in_maps = []
    for c in range(NCORES):
        inc_c = inc[:, c * E_SH:(c + 1) * E_SH]
        inc_t = np.ascontiguousarray(
            inc_c.reshape(T, 128, E_SH).transpose(1, 0, 2))
        in_maps.append(dict(
            inc=inc_t,
            efeat=np.ascontiguousarray(ef_scaled[c * E_SH:(c + 1) * E_SH]),
            **common))
    return in_maps


def kernel(node_feats, edge_feats, inc_mat, W_att, W_proj, alpha,
           ec_W_att, ec_W_proj, ec_b_proj, fc_W, fc_b,
           mode="f16", trace=False):
    alpha_f = float(np.asarray(alpha))
    nc = get_nc(alpha_f, mode)
    in_maps = make_in_maps(node_feats, edge_feats, inc_mat, W_att, W_proj,
                           alpha_f, ec_W_att, ec_W_proj, fc_W, mode=mode)
    res = run_bass_kernel_spmd(nc, in_maps, list(range(NCORES)), trace=trace)
    kernel.last_results = res
    outs = np.stack([np.asarray(res.results[c]["out"]).reshape(8)
                     for c in range(NCORES)]).astype(np.float64)
    r, z, m = outs[:, 0:NCAT], outs[:, 3], outs[:, 4]
    w = np.exp(m - m.max())
    const = (np.asarray(ec_b_proj, np.float64) @
             np.asarray(fc_W, np.float64)) + np.asarray(fc_b, np.float64)
    logits = (w @ r) / float(w @ z) + const
    return logits.astype(np.float32)


# revision 8
# speedup vs baseline: 3.0296x; 1.1619x over previous
"""HGConv fused kernel for one TRN2 chip (8 NeuronCores), SPMD via Bass/Tile.

Hardcoded for M=16384 nodes, E=4096 hyperedges, D=300, N_CAT=3, 8 cores.

Strategy (edge-sharded, zero collectives):
  - Core c owns hyperedges [512c, 512(c+1)).  It loads the FULL node
    matrix X (16384, 300) plus its own 512-column slice of inc, both in
    fp16 (halves HBM traffic; final rel err ~1e-3), and computes
    IX = inc_c.T @ X with the full m=16384 contraction locally — no
    ReduceScatter, no AllGather, no cross-core barrier at all.
  - Host pre-tiles both operands to [p, ...] layout so every DMA is a
    contiguous multi-KB line per partition.  inc streams et-major
    (one 128-edge tile of output at a time) through a 5-deep SBUF pool
    on two DMA queues, with small leading chunks so the PE starts
    within ~15 us; X is resident, loaded chunk-wise on a third queue.
  - The per-e-tile tail (transposes, edge_att = IX @ W_att, softmax
    over d, ef = (IX*attn) @ W_proj, residual mix) is pipelined behind
    the NEXT e-tile's accumulation matmuls, so only the global tail
    (edge-score softmax stats, pooling p2, final projection) is
    exposed.  Tail matmul operands are cast to fp16 (1 PE cycle/row
    instead of 4 for fp32).
  - ec_W_proj @ fc_W is folded on the host into one (300, 3) weight.
    Each core outputs 8 floats: [r(3), z, m, pad]; the host combines
    the 8 partials (global softmax over edges = weighted sum of the
    locally-normalized partials) and adds ec_b @ fc_W + fc_b.
"""

import sys

for _p in ("/opt/trn_rl_repo", "/opt/pypackages"):
    if _p not in sys.path:
        sys.path.append(_p)

import numpy as np

import concourse.bacc as bacc
import concourse.tile as tile
from concourse import masks, mybir
from concourse.bass_utils import run_bass_kernel_spmd

F32 = mybir.dt.float32
F32R = mybir.dt.float32r
F16 = mybir.dt.float16
BF16 = mybir.dt.bfloat16
AX = mybir.AxisListType
OP = mybir.AluOpType
AF = mybir.ActivationFunctionType

NCORES = 8
M, E, D, NCAT = 16384, 4096, 300, 3
E_SH = E // NCORES          # 512 edges per core
ET = E_SH // 128            # 4 e-tiles per core
T = M // 128                # 128 m-tiles (full contraction on every core)
DCH = (128, 128, 44)        # d split into partition chunks
DOF = (0, 128, 256)
X_CH = (8, 8, 16, 16, 16, 16, 16, 16, 16)       # x chunk sizes (t-tiles)
INC_CH = {0: (32, 32, 32, 32), 1: (64, 64), 2: (64, 64), 3: (64, 64)}


def _build(alpha: float, mode: str):
    nc = bacc.Bacc("TRN2", target_bir_lowering=False, debug=False,
                   num_devices=NCORES)
    in_dt = F16 if mode == "f16" else BF16
    x_d = nc.dram_tensor("x", [128, T, D], in_dt, kind="ExternalInput")
    inc_d = nc.dram_tensor("inc", [128, ET, T, 128], in_dt,
                           kind="ExternalInput")
    ef_d = nc.dram_tensor("efeat", [E_SH, D], F32, kind="ExternalInput")
    watt_d = nc.dram_tensor("watt", [D, D], in_dt, kind="ExternalInput")
    wproj_d = nc.dram_tensor("wproj", [D, D], in_dt, kind="ExternalInput")
    ecwatt_d = nc.dram_tensor("ecwatt", [D, 1], in_dt, kind="ExternalInput")
    w2_d = nc.dram_tensor("w2", [D, NCAT], F32, kind="ExternalInput")
    out_d = nc.dram_tensor("out", [1, 8], F32, kind="ExternalOutput")

    def mm(out, lhsT, rhs, start, stop):
        nc.tensor.matmul(out, lhsT, rhs, start=start, stop=stop)

    with tile.TileContext(nc) as tc, \
         tc.tile_pool(name="sb", bufs=1) as sb, \
         tc.tile_pool(name="xp", bufs=1) as xp, \
         tc.tile_pool(name="incp", bufs=5) as incp, \
         tc.tile_pool(name="pacc", bufs=2, space="PSUM") as pacc, \
         tc.tile_pool(name="pp", bufs=6, space="PSUM") as pp:

        # ---------- X chunks (sync queue), weights interleaved ----------
        x_tiles = []
        x_off = []
        off = 0
        for ci, sz in enumerate(X_CH):
            xt = xp.tile([128, sz, D], in_dt, tag=f"x{ci}", name=f"x{ci}")
            nc.sync.dma_start(xt[:], x_d[:, off:off + sz, :])
            x_tiles.append(xt)
            x_off.append(off)
            off += sz

        def x_at(t):
            for ci in range(len(X_CH) - 1, -1, -1):
                if x_off[ci] <= t:
                    return x_tiles[ci][:, t - x_off[ci], :]

        watt_sb = sb.tile([128, 3, D], in_dt)
        wproj_sb = sb.tile([128, 3, D], in_dt)
        ecwatt_sb = sb.tile([128, 3, 1], in_dt)
        w2_sb = sb.tile([128, 3, NCAT], F32)
        for i, (c, o) in enumerate(zip(DCH, DOF)):
            nc.sync.dma_start(watt_sb[:c, i, :], watt_d[o:o + c, :])
            nc.sync.dma_start(wproj_sb[:c, i, :], wproj_d[o:o + c, :])
            nc.sync.dma_start(ecwatt_sb[:c, i, :], ecwatt_d[o:o + c, :])
            nc.sync.dma_start(w2_sb[:c, i, :], w2_d[o:o + c, :])
        efeat_sb = sb.tile([128, ET, D], F32)   # pre-scaled by alpha on host
        nc.sync.dma_start(efeat_sb[:],
                          ef_d.ap().rearrange("(t p) d -> p t d", p=128))
        ident = sb.tile([128, 128], F32)
        masks.make_identity(nc, ident[:])

        # ---------- working tiles for the tail ----------
        ix_sb = sb.tile([128, ET, D], F32)
        ef_sb = sb.tile([128, ET, D], F32)
        ef2_sb = sb.tile([128, ET, D], F32)
        ixT_sb = sb.tile([128, 3, E_SH], in_dt)
        efT_sb = sb.tile([128, 3, E_SH], in_dt)
        ef2T_sb = sb.tile([128, 3, E_SH], in_dt)
        stat_sb = sb.tile([128, ET, 4], F32)

        def tr_chunks(src_ap_fn, dstT_sb, et, use_vec):
            # (128e, 300d) -> 3 d-part chunks of (c, 128e), cast to in_dt
            for i, (c, o) in enumerate(zip(DCH, DOF)):
                tp = pp.tile([128, 128], F32, tag="ps", name=f"tp{et}_{i}")
                nc.tensor.transpose(tp[:c, :128], src_ap_fn(o, c), ident[:])
                dst = dstT_sb[:c, i, et * 128:(et + 1) * 128]
                if use_vec:
                    nc.vector.tensor_copy(dst, tp[:c, :128])
                else:
                    nc.scalar.copy(dst, tp[:c, :128])

        def early_tail(et):
            # IX psum -> sbuf; edge_att = IX @ W_att; softmax over d;
            # ef = IX * attn
            nc.vector.tensor_copy(ix_sb[:, et, :], accs[et][:])
            tr_chunks(lambda o, c: ix_sb[:, et, o:o + c], ixT_sb, et, False)
            att = pp.tile([128, D], F32, tag="ps", name=f"att{et}")
            for i, c in enumerate(DCH):
                mm(att[:], ixT_sb[:c, i, et * 128:(et + 1) * 128],
                   watt_sb[:c, i, :], start=(i == 0), stop=(i == 2))
            nmax = stat_sb[:, et, 0:1]
            nc.vector.tensor_reduce(nmax, att[:], axis=AX.X, op=OP.max,
                                    negate=True)
            ex = pp.tile([128, D], F32, tag="ps", name=f"ex{et}")
            rsum = stat_sb[:, et, 1:2]
            nc.scalar.activation(ex[:], att[:], AF.Exp, bias=nmax,
                                 scale=1.0, accum_out=rsum)
            rcp = stat_sb[:, et, 2:3]
            nc.vector.reciprocal(rcp, rsum)
            nc.vector.scalar_tensor_tensor(
                ef_sb[:, et, :], ex[:], rcp, ix_sb[:, et, :],
                op0=OP.mult, op1=OP.mult)

        def late_tail(et):
            # ef2 = alpha*edge_feats + (1-alpha)*(ef @ W_proj); transpose
            tr_chunks(lambda o, c: ef_sb[:, et, o:o + c], efT_sb, et, True)
            prj = pp.tile([128, D], F32, tag="ps", name=f"prj{et}")
            for i, c in enumerate(DCH):
                mm(prj[:], efT_sb[:c, i, et * 128:(et + 1) * 128],
                   wproj_sb[:c, i, :], start=(i == 0), stop=(i == 2))
            nc.vector.scalar_tensor_tensor(
                ef2_sb[:, et, :], prj[:], float(1.0 - alpha),
                efeat_sb[:, et, :], op0=OP.mult, op1=OP.add)
            tr_chunks(lambda o, c: ef2_sb[:, et, o:o + c], ef2T_sb, et, False)

        # ---------- phase 1 (et-major) with pipelined per-et tail ----------
        accs = []
        for et in range(ET):
            acc = pacc.tile([128, D], F32, tag="acc", name=f"acc{et}")
            accs.append(acc)
            tlo = 0
            for ci, tc_sz in enumerate(INC_CH[et]):
                inc_sb = incp.tile([128, 64, 128], in_dt, tag="inc",
                                   name=f"inc{et}_{ci}")
                eng = nc.scalar if ci % 2 == 0 else nc.gpsimd
                eng.dma_start(inc_sb[:, 0:tc_sz, :],
                              inc_d[:, et, tlo:tlo + tc_sz, :])
                for tt in range(tlo, tlo + tc_sz):
                    mm(acc[:], inc_sb[:, tt - tlo, :], x_at(tt),
                       start=(tt == 0), stop=(tt == T - 1))
                tlo += tc_sz
            if et >= 1:
                late_tail(et - 1)
            early_tail(et)
        late_tail(ET - 1)

        # ---------- global tail ----------
        # scores (1, 512); locally stabilized exp weights
        sc = pp.tile([1, E_SH], F32, tag="ps", name="sc")
        for i, c in enumerate(DCH):
            mm(sc[:], ecwatt_sb[:c, i, :], ef2T_sb[:c, i, :],
               start=(i == 0), stop=(i == 2))
        one_sb = sb.tile([1, 520], F32)
        nloc = one_sb[:, 512:513]
        nc.vector.tensor_reduce(nloc, sc[:], axis=AX.X, op=OP.max,
                                negate=True)
        expw = one_sb[:, 0:512]
        zloc = one_sb[:, 513:514]
        nc.scalar.activation(expw, sc[:], AF.Exp, bias=nloc, scale=1.0,
                             accum_out=zloc)
        mloc = one_sb[:, 514:515]
        nc.scalar.mul(mloc, nloc, -1.0)

        expcol_sb = sb.tile([128, ET], F32)
        for et in range(ET):
            tc1 = pp.tile([128, 1], F32, tag="ps", name=f"tc1_{et}")
            nc.tensor.transpose(tc1[:], expw[0:1, et * 128:(et + 1) * 128],
                                ident[0:1, 0:1])
            nc.scalar.copy(expcol_sb[:, et:et + 1], tc1[:])

        # p2 = sum_e w_e * ef2[e, :]  (pooling with unnormalized weights)
        p2 = pp.tile([1, D], F32, tag="ps", name="p2")
        for et in range(ET):
            mm(p2[:], expcol_sb[:, et:et + 1], ef2_sb[:, et, :],
               start=(et == 0), stop=(et == ET - 1))
        p2_sb = sb.tile([1, D], F32)
        nc.scalar.copy(p2_sb[:], p2[:])

        # r = p2 @ (ec_W_proj @ fc_W)   (w2 folded on host)
        pcol_sb = sb.tile([128, 3, 1], F32)
        for i, (c, o) in enumerate(zip(DCH, DOF)):
            tpc = pp.tile([128, 1], F32, tag="ps", name=f"tpc{i}")
            nc.tensor.transpose(tpc[:c, :], p2_sb[0:1, o:o + c],
                                ident[0:1, 0:1])
            nc.scalar.copy(pcol_sb[:c, i, :], tpc[:c, :])
        r = pp.tile([1, NCAT], F32, tag="ps", name="r")
        for i, c in enumerate(DCH):
            mm(r[:], pcol_sb[:c, i, :], w2_sb[:c, i, :],
               start=(i == 0), stop=(i == 2))

        out_sb = sb.tile([1, 8], F32)
        nc.vector.memset(out_sb[:], 0.0)
        nc.scalar.copy(out_sb[:, 0:NCAT], r[:])
        nc.scalar.copy(out_sb[:, 3:4], zloc)
        nc.scalar.copy(out_sb[:, 4:5], mloc)
        nc.sync.dma_start(out_d[:], out_sb[:])

    nc.compile()
    return nc


_CACHE = {}


def get_nc(alpha: float, mode: str = "f16"):
    key = (alpha, mode)
    if key not in _CACHE:
        _CACHE[key] = _build(alpha, mode)
    return _CACHE[key]


def make_in_maps(node_feats, edge_feats, inc_mat, W_att, W_proj, alpha,
                 ec_W_att, ec_W_proj, fc_W, mode="f16"):
    if mode == "f16":
        ndt = np.float16
    else:
        import ml_dtypes
        ndt = ml_dtypes.bfloat16
    wdt = lambda a: np.ascontiguousarray(np.asarray(a, np.float32).astype(ndt))
    x = np.asarray(node_feats, np.float32).astype(ndt)
    xt = np.ascontiguousarray(x.reshape(T, 128, D).transpose(1, 0, 2))
    inc = np.asarray(inc_mat, np.float32).astype(ndt)
    ef_scaled = np.asarray(edge_feats, np.float32) * np.float32(alpha)
    w2 = (np.asarray(ec_W_proj, np.float64) @
          np.asarray(fc_W, np.float64)).astype(np.float32)
    common = dict(x=xt, watt=wdt(W_att), wproj=wdt(W_proj),
                  ecwatt=wdt(np.asarray(ec_W_att).reshape(D, 1)),
                  w2=np.ascontiguousarray(w2))
    in_maps = []
    for c in range(NCORES):
        inc_c = inc[:, c * E_SH:(c + 1) * E_SH]
        # [p, et, t, e8]: m = t*128 + p, e_local = et*128 + e8
        inc_t = np.ascontiguousarray(
            inc_c.reshape(T, 128, ET, 128).transpose(1, 2, 0, 3))
        in_maps.append(dict(
            inc=inc_t,
            efeat=np.ascontiguousarray(ef_scaled[c * E_SH:(c + 1) * E_SH]),
            **common))
    return in_maps


def kernel(node_feats, edge_feats, inc_mat, W_att, W_proj, alpha,
           ec_W_att, ec_W_proj, ec_b_proj, fc_W, fc_b,
           mode="f16", trace=False):
    alpha_f = float(np.asarray(alpha))
    nc = get_nc(alpha_f, mode)
    in_maps = make_in_maps(node_feats, edge_feats, inc_mat, W_att, W_proj,
                           alpha_f, ec_W_att, ec_W_proj, fc_W, mode=mode)
    res = run_bass_kernel_spmd(nc, in_maps, list(range(NCORES)), trace=trace)
    kernel.last_results = res
    outs = np.stack([np.asarray(res.results[c]["out"]).reshape(8)
                     for c in range(NCORES)]).astype(np.float64)
    r, z, m = outs[:, 0:NCAT], outs[:, 3], outs[:, 4]
    w = np.exp(m - m.max())
    const = (np.asarray(ec_b_proj, np.float64) @
             np.asarray(fc_W, np.float64)) + np.asarray(fc_b, np.float64)
    logits = (w @ r) / float(w @ z) + const
    return logits.astype(np.float32)
